# revision 4
# baseline (speedup 1.0000x reference)
"""DCNet Trainium2 kernel — data-parallel over 8 NeuronCores.

Model (per reference):
    hidden = relu(relu(x @ W1 + b1) @ W2 + b2)                    # [B, 512]
    tb     = treat_basis(t)                                       # [B, 25]
    h1     = relu(einsum('bi,iod,bd->bo', hidden, dfc1_w, tb) + tb @ dfc1_b.T)
    Q      = einsum('bi,iod,bd->bo', h1, dfc2_w, tb) + tb @ dfc2_b.T
    returns (Q [B,1], hidden [B,512])

Strategy: shard batch B=16384 over 8 cores (2048 rows each), replicate weights.
Per core, activations are kept transposed ([feature, batch]) so the contraction
dim lands on SBUF partitions for the PE. The dominant op (dfc1, 215 GFLOP total)
runs as, per 128-row batch tile and per spline-basis index d:
    psum_d[b,o] += hiddenT[i_chunk, b].T @ dfc1_w[i_chunk, :, d]   (4 K-chunks)
    acc[b,o]    = psum_d * tb[b,d] + acc          (fused DVE scalar_tensor_tensor)
Q is then rowsum(relu(acc) * u) where u = tb @ dfc2_w[:,0,:].T computed by PE.

Matmul operands are bf16 (fp32 PSUM accumulation); tb scaling stays fp32.
"""
import numpy as np
import ml_dtypes

B = 16384
NCORES = 8
BC = B // NCORES          # 2048 rows per core
NBT = BC // 128           # 16 batch tiles per core
COV = 256
H = 512
DD = 25
KNOTS = [0.33, 0.66]
DEGREE = 2

_BF16 = ml_dtypes.bfloat16

_CACHE = {}


def _treat_basis(t):
    """Truncated power basis + kron, matching the reference (fp32 numpy)."""
    t = t.astype(np.float32)
    knots = np.asarray([KNOTS, KNOTS], dtype=np.float32)              # [2, K]
    powers = np.stack([t ** p for p in range(DEGREE + 1)], axis=-1)   # [B, 2, 3]
    rel = np.maximum(t[..., None] - knots[None], 0.0) ** DEGREE       # [B, 2, 2]
    basis = np.concatenate([powers, rel], axis=-1)                    # [B, 2, 5]
    tb = np.einsum('bi,bj->bij', basis[:, 0], basis[:, 1])
    return tb.reshape(t.shape[0], -1)                                 # [B, 25]


def _build_nc():
    if "nc" in _CACHE:
        return _CACHE["nc"]
    from concourse import bacc, mybir
    import concourse.tile as tile

    BF16 = mybir.dt.bfloat16
    F32 = mybir.dt.float32
    AF = mybir.ActivationFunctionType
    OP = mybir.AluOpType

    nc = bacc.Bacc("TRN2", target_bir_lowering=False, debug=False,
                   num_devices=NCORES)

    xt_d = nc.dram_tensor("xt", [2, 128, BC], BF16, kind="ExternalInput").ap()
    w1_d = nc.dram_tensor("w1", [2, 128, H], BF16, kind="ExternalInput").ap()
    w2_d = nc.dram_tensor("w2", [4, 128, H], BF16, kind="ExternalInput").ap()
    b1_d = nc.dram_tensor("b1c", [128, 4], F32, kind="ExternalInput").ap()
    b2_d = nc.dram_tensor("b2c", [128, 4], F32, kind="ExternalInput").ap()
    wt_d = nc.dram_tensor("wt", [4, 128, DD, H], BF16, kind="ExternalInput").ap()
    d1b_d = nc.dram_tensor("d1bT", [DD, H], BF16, kind="ExternalInput").ap()
    tbp_d = nc.dram_tensor("tbp", [128, NBT, DD], F32, kind="ExternalInput").ap()
    tbt_d = nc.dram_tensor("tbt", [DD, BC], BF16, kind="ExternalInput").ap()
    w2d_d = nc.dram_tensor("w2d", [DD, 516], BF16, kind="ExternalInput").ap()

    hid_d = nc.dram_tensor("hid", [4, 128, BC], F32, kind="ExternalOutput").ap()
    q_d = nc.dram_tensor("q", [128, NBT], F32, kind="ExternalOutput").ap()

    # d-group schedule for the dfc1 accumulation: 26 psum slots
    # (1 bias + 25 basis indices) in groups of <=4 (one [128, 2048] psum tile
    # = 4 banks each, double-buffered = all 8 banks).
    slots = [None] + list(range(DD))     # None = dfc1 bias matmul
    groups = [slots[i:i + 4] for i in range(0, 26, 4)]   # 6*4 + 2

    with tile.TileContext(nc) as tc:
        with (
            tc.tile_pool(name="const", bufs=1) as cp,
            tc.tile_pool(name="work", bufs=2) as wp,
        ):
            xt = cp.tile([128, 2, BC], BF16)
            for ic in range(2):
                nc.sync.dma_start(xt[:, ic, :], xt_d[ic])
            w1 = cp.tile([128, 2, H], BF16)
            for ic in range(2):
                nc.sync.dma_start(w1[:, ic, :], w1_d[ic])
            w2 = cp.tile([128, 4, H], BF16)
            for ic in range(4):
                nc.sync.dma_start(w2[:, ic, :], w2_d[ic])
            b1s = cp.tile([128, 4], F32)
            nc.sync.dma_start(b1s[:], b1_d)
            b2s = cp.tile([128, 4], F32)
            nc.sync.dma_start(b2s[:], b2_d)
            wt = []
            for ic in range(4):
                w_ic = cp.tile([128, DD, H], BF16, tag=f"wt{ic}")
                nc.sync.dma_start(w_ic[:], wt_d[ic])
                wt.append(w_ic)
            d1b = cp.tile([DD, H], BF16)
            nc.sync.dma_start(d1b[:], d1b_d)
            tbp = cp.tile([128, NBT, DD], F32)
            nc.sync.dma_start(tbp[:], tbp_d)
            tbt = cp.tile([DD, BC], BF16)
            nc.sync.dma_start(tbt[:], tbt_d)
            w2d = cp.tile([DD, 516], BF16)
            nc.sync.dma_start(w2d[:], w2d_d)

            h1t = cp.tile([128, 4, BC], BF16)
            ht = cp.tile([128, 4, BC], BF16)
            qout = cp.tile([128, NBT], F32)

            # ---- phase 1: representation MLP (transposed activations) ----
            with tc.tile_pool(name="psum1", bufs=4, space="PSUM") as pp1:
                with nc.named_scope("mlp1"):
                    for ot in range(4):
                        osl = slice(ot * 128, (ot + 1) * 128)
                        for bc_i in range(4):
                            bsl = slice(bc_i * 512, (bc_i + 1) * 512)
                            ps = pp1.tile([128, 512], F32, tag="ps1")
                            for ic in range(2):
                                nc.tensor.matmul(ps[:], w1[:, ic, osl],
                                                 xt[:, ic, bsl],
                                                 start=(ic == 0), stop=(ic == 1))
                            nc.scalar.activation(h1t[:, ot, bsl], ps[:], AF.Relu,
                                                 bias=b1s[:, ot:ot + 1])
                with nc.named_scope("mlp2"):
                    for ot in range(4):
                        osl = slice(ot * 128, (ot + 1) * 128)
                        for bc_i in range(4):
                            bsl = slice(bc_i * 512, (bc_i + 1) * 512)
                            ps = pp1.tile([128, 512], F32, tag="ps1")
                            for ic in range(4):
                                nc.tensor.matmul(ps[:], w2[:, ic, osl],
                                                 h1t[:, ic, bsl],
                                                 start=(ic == 0), stop=(ic == 3))
                            nc.scalar.activation(ht[:, ot, bsl], ps[:], AF.Relu,
                                                 bias=b2s[:, ot:ot + 1])
                            hs = wp.tile([128, 512], F32, tag="hstage")
                            nc.scalar.activation(hs[:], ps[:], AF.Relu,
                                                 bias=b2s[:, ot:ot + 1])
                            nc.sync.dma_start(hid_d[ot][:, bsl], hs[:])

            # ---- phase 2: dynamic FC layers ----
            with tc.tile_pool(name="psum2", bufs=2, space="PSUM") as pp2:
                with nc.named_scope("dfc"):
                    for bt in range(NBT):
                        bsl = slice(bt * 128, (bt + 1) * 128)
                        acc = wp.tile([128, 512], F32, tag="acc")
                        for group in groups:
                            ps = pp2.tile([128, 2048], F32, tag="ps2")
                            for ic in range(4):
                                for j, d in enumerate(group):
                                    psl = ps[:, j * 512:(j + 1) * 512]
                                    if d is None:
                                        if ic == 0:
                                            nc.tensor.matmul(
                                                psl, tbt[:, bsl], d1b[:],
                                                start=True, stop=True)
                                    else:
                                        nc.tensor.matmul(
                                            psl, ht[:, ic, bsl], wt[ic][:, d, :],
                                            start=(ic == 0), stop=(ic == 3))
                            for j, d in enumerate(group):
                                psl = ps[:, j * 512:(j + 1) * 512]
                                if d is None:
                                    nc.vector.tensor_copy(acc[:], psl)
                                else:
                                    nc.vector.scalar_tensor_tensor(
                                        acc[:], psl, tbp[:, bt, d:d + 1], acc[:],
                                        OP.mult, OP.add)
                        # h1 = relu(acc); Q = rowsum(h1 * u) + u_bias
                        h1b = wp.tile([128, 512], BF16, tag="h1b")
                        nc.scalar.activation(h1b[:], acc[:], AF.Relu)
                        ups = pp2.tile([128, 2048], F32, tag="ps2")
                        nc.tensor.matmul(ups[:, 0:512], tbt[:, bsl],
                                         w2d[:, 0:512], start=True, stop=True)
                        nc.tensor.matmul(ups[:, 512:513], tbt[:, bsl],
                                         w2d[:, 512:513], start=True, stop=True)
                        vt = wp.tile([128, 512], F32, tag="vt")
                        qraw = wp.tile([128, 1], F32, tag="qraw")
                        nc.vector.scalar_tensor_tensor(
                            vt[:], ups[:, 0:512], 1.0, h1b[:],
                            OP.mult, OP.mult, accum_out=qraw[:])
                        nc.vector.tensor_add(qout[:, bt:bt + 1], qraw[:],
                                             ups[:, 512:513])
                    nc.sync.dma_start(q_d[:], qout[:])

    nc.finalize()
    _CACHE["nc"] = nc
    return nc


def _prepare(t, x, W1, b1, W2, b2, dfc1_w, dfc1_b, dfc2_w, dfc2_b):
    t = np.asarray(t, dtype=np.float32)
    x = np.asarray(x, dtype=np.float32)
    W1 = np.asarray(W1, dtype=np.float32)
    b1 = np.asarray(b1, dtype=np.float32)
    W2 = np.asarray(W2, dtype=np.float32)
    b2 = np.asarray(b2, dtype=np.float32)
    dfc1_w = np.asarray(dfc1_w, dtype=np.float32)
    dfc1_b = np.asarray(dfc1_b, dtype=np.float32)
    dfc2_w = np.asarray(dfc2_w, dtype=np.float32)
    dfc2_b = np.asarray(dfc2_b, dtype=np.float32)

    # replicated weights (host-side relayouts)
    w1 = np.ascontiguousarray(W1.reshape(2, 128, H)).astype(_BF16)
    w2 = np.ascontiguousarray(W2.reshape(4, 128, H)).astype(_BF16)
    b1c = np.ascontiguousarray(b1.reshape(4, 128).T)
    b2c = np.ascontiguousarray(b2.reshape(4, 128).T)
    wt = np.ascontiguousarray(
        dfc1_w.reshape(4, 128, H, DD).transpose(0, 1, 3, 2)).astype(_BF16)
    d1bT = np.ascontiguousarray(dfc1_b.T).astype(_BF16)          # [25, 512]
    w2d = np.zeros((DD, 516), np.float32)
    w2d[:, :512] = dfc2_w[:, 0, :].T
    w2d[:, 512] = dfc2_b[0]
    w2d = w2d.astype(_BF16)

    tb = _treat_basis(t)                                          # [B, 25] f32

    in_maps = []
    for c in range(NCORES):
        rs = slice(c * BC, (c + 1) * BC)
        xs = x[rs]
        tbs = tb[rs]
        in_maps.append(dict(
            xt=np.ascontiguousarray(xs.T.reshape(2, 128, BC)).astype(_BF16),
            w1=w1, w2=w2, b1c=b1c, b2c=b2c, wt=wt, d1bT=d1bT,
            tbp=np.ascontiguousarray(tbs.reshape(NBT, 128, DD).transpose(1, 0, 2)),
            tbt=np.ascontiguousarray(tbs.T).astype(_BF16),
            w2d=w2d,
        ))
    return in_maps


def _gather(res):
    Q = np.empty((B, 1), np.float32)
    hidden = np.empty((B, H), np.float32)
    for c in range(NCORES):
        rs = slice(c * BC, (c + 1) * BC)
        hid = res.results[c]["hid"]                # [4, 128, BC]
        hidden[rs] = hid.reshape(H, BC).T
        q = res.results[c]["q"]                    # [128, NBT]
        Q[rs, 0] = q.T.reshape(BC)
    return (Q, hidden)


def kernel(t, x, W1, b1, W2, b2, dfc1_w, dfc1_b, dfc2_w, dfc2_b):
    from concourse.bass_utils import run_bass_kernel_spmd

    in_maps = _prepare(t, x, W1, b1, W2, b2, dfc1_w, dfc1_b, dfc2_w, dfc2_b)
    nc = _build_nc()
    res = run_bass_kernel_spmd(nc, in_maps, core_ids=list(range(NCORES)))
    return _gather(res)
